# revision 28
# baseline (speedup 1.0000x reference)
"""Trainium2 Bass kernel for DynamicResidualStageWrapper (18-block MLP stage
with channel-gated anchor routing), data-parallel over batch across 8 cores.

Contract: kernel(**inputs) takes FULL unsharded inputs (as numpy arrays, keyed
as in reference.setup_inputs()) and returns the FULL output [32,14,14,512].

Per-core layout: activations live transposed as [C=512, tokens=784] split into
4 partition-tiles [128, 784] bf16; tokens are (sample b, position hw). Block
weights [cin, cout] are the natural lhsT for out[cout, tok] = W.T @ X so there
are no transposes anywhere. Everything runs in bf16 (tolerance 2e-2; measured
~5e-3): same PE rate as f32r but half the DMA/SBUF traffic and 2x DVE rate.

Key scheduling ideas vs a straightforward version:
- startup: ACT table loads fire before the input-DMA flood (else they starve
  behind it and block every block-0 gelu); descriptor generation is ~0.7us
  serial per dma_start, so transfers are consolidated (one rearranged-AP call
  per weight block) and spread over the sync/scalar/gpsimd queues; dummy
  matmuls on zeroed tiles warm the PE HAM clock-gate so real matmuls run at
  2.4 GHz from the first block.
- routers: the gate-independent base term (Xn + gamma*a2) is emitted as
  per-sample slices whose accumulators give the mean-pool for free (corrected
  by precomputed gamma*pooled(a2)); corrections are applied sample-major so
  the next block's matmuls (sliced per-sample, N=196) restart as soon as the
  first sample's gates land; the DVE's scalar_tensor_tensor is stuck in
  1x mode, so half of each sample's corrections run as ACT Identity
  (per-partition gate scale) + 2x-mode DVE adds. Dummy matmuls keep HAM warm
  across the router bubble.
- block 17 streams its output to HBM per (k, sample-pair) as corrections
  complete, across both HWDGE queues, shrinking the tail.
"""

import numpy as np

import concourse.bacc as bacc
import concourse.mybir as mybir
import concourse.tile as tile
from concourse.bass_utils import run_bass_kernel_spmd

# ---- problem constants (hardcoded per spec) ----
NUM_BLOCKS = 18
ANCHOR_IDX = (1, 4, 9)
TARGET_IDX = (11, 14, 17)
POST_ROUTER = (12, 15)     # blocks whose input trickles in per-sample
C = 512
HID = 128
A = 3
B, H, W = 32, 14, 14
N_CORES = 8
BL = B // N_CORES          # 4 samples per core
HW = H * W                 # 196 positions per sample
T = BL * HW                # 784 tokens per core
KT = C // 128              # 4 channel tiles
NCH = 2                    # token chunks per row (normal blocks)
CH = T // NCH              # 392 tokens per chunk

F32 = mybir.dt.float32
BF16 = mybir.dt.bfloat16
GELU = mybir.ActivationFunctionType.Gelu_apprx_tanh
TANH = mybir.ActivationFunctionType.Tanh
IDENT = mybir.ActivationFunctionType.Identity
_cached = {}


def build_program():
    """Build the per-core Bass/Tile program (same program on all 8 cores)."""
    nc = bacc.Bacc(trn_type="TRN2", target_bir_lowering=False, debug=False)

    xT = nc.dram_tensor("xT", [C, T], BF16, kind="ExternalInput").ap()
    wd = nc.dram_tensor("wd", [NUM_BLOCKS, C, C], BF16, kind="ExternalInput").ap()
    bias_cols = nc.dram_tensor("bias_cols", [128, NUM_BLOCKS * KT], F32,
                               kind="ExternalInput").ap()
    fc1w = nc.dram_tensor("fc1w", [128, A * KT * 128], BF16, kind="ExternalInput").ap()
    fc1b = nc.dram_tensor("fc1b", [128, A], F32, kind="ExternalInput").ap()
    fc2w = nc.dram_tensor("fc2w", [128, A * A * C], BF16, kind="ExternalInput").ap()
    fc2bias = nc.dram_tensor("fc2bias", [128, A * A * KT * BL], F32,
                             kind="ExternalInput").ap()
    # cols 0..A-1: gamma_t; cols A..2A-1: -gamma_t (bf16 so DVE scalar-tensor
    # ops keep their 2x bf16 mode); f32 copy for ops that require f32 scalars
    gbc = nc.dram_tensor("gbc", [128, 2 * A], BF16, kind="ExternalInput").ap()
    gbc32 = nc.dram_tensor("gbc32", [128, A], F32, kind="ExternalInput").ap()
    outT = nc.dram_tensor("outT", [C, T], BF16, kind="ExternalOutput").ap()

    anchor_of = {b: i for i, b in enumerate(ANCHOR_IDX)}
    target_of = {b: i for i, b in enumerate(TARGET_IDX)}

    with tile.TileContext(nc) as tc:
        with (
            tc.tile_pool(name="const", bufs=1) as cpool,
            tc.tile_pool(name="wpool", bufs=6) as wpool,
            tc.tile_pool(name="xpool", bufs=4) as xpool,
            tc.tile_pool(name="apool", bufs=1) as apool,
            tc.tile_pool(name="rpool", bufs=3) as rpool,
            tc.tile_pool(name="ppool", bufs=6, space="PSUM") as ppool,
            tc.tile_pool(name="fcps", bufs=1, space="PSUM") as fcps,
            tc.tile_pool(name="dpool", bufs=1, space="PSUM") as dpool,
        ):
            # ---- HAM warm-up: zeroed bf16 tiles + dummy matmuls keep the PE
            # busy during the startup DMA wait so real matmuls start at 2.4GHz
            # (memsets on gpsimd: its queue starts ~1us before the vector one)
            zw = cpool.tile([128, 128], BF16, name="zw")
            zx = cpool.tile([128, 512], BF16, name="zx")
            # gelu ACT-table load MUST fire before the input-DMA flood starts,
            # or its TDRAM DMA gets starved and blocks every block-0 gelu.
            # zw/zx memsets go on the (otherwise idle) vector engine so the
            # gpsimd queue reaches its x-piece descriptor gens sooner.
            warm = cpool.tile([128, 1], F32, name="warm")
            nc.gpsimd.memset(warm[:], 0.0)
            nc.scalar.activation(warm[:], warm[:], GELU)
            nc.vector.memset(zw[:], 0.0)
            nc.vector.memset(zx[:], 0.0)
            _dummy_mms(nc, dpool, zw, zx, n512=4)

            # ---- startup DMAs.  Descriptor generation costs ~0.7us SERIAL
            # per dma_start on the issuing queue, so: the scalar/ACT queue
            # does ONLY the table warms (+tiny bias), the sync queue carries
            # w0/w1 + two x chunk-0 pieces, gpsimd (SWDGE) the other two.
            nc.scalar.activation(warm[:], warm[:], TANH)
            nc.scalar.activation(warm[:], warm[:], IDENT)
            X = []
            for k in range(KT):
                xt = xpool.tile([128, T], BF16, tag=f"x{k}", name=f"xin{k}")
                X.append(xt)
            wtiles = {}
            w_t = wpool.tile([128, KT * C], BF16, tag="w", name="w0")
            nc.sync.dma_start(w_t[:].rearrange("p (k c) -> p k c", k=KT),
                              wd[0].rearrange("(k p) c -> p k c", p=128))
            wtiles[0] = w_t
            for k in range(KT):
                eng = nc.sync if k < 2 else nc.gpsimd
                eng.dma_start(X[k][:, 0:CH], xT[k * 128:(k + 1) * 128, 0:CH])
            bias_t = cpool.tile([128, NUM_BLOCKS * KT], F32, name="bias_t")
            nc.scalar.dma_start(bias_t[:], bias_cols[:])
            w_t = wpool.tile([128, KT * C], BF16, tag="w", name="w1")
            nc.sync.dma_start(w_t[:].rearrange("p (k c) -> p k c", k=KT),
                              wd[1].rearrange("(k p) c -> p k c", p=128))
            wtiles[1] = w_t
            for k in range(KT):
                eng = nc.sync if k < 2 else nc.gpsimd
                eng.dma_start(X[k][:, CH:T], xT[k * 128:(k + 1) * 128, CH:T])
            for i in (2, 3):
                w_t = wpool.tile([128, KT * C], BF16, tag="w", name=f"w{i}")
                eng = nc.sync if i == 2 else nc.scalar
                eng.dma_start(w_t[:].rearrange("p (k c) -> p k c", k=KT),
                              wd[i].rearrange("(k p) c -> p k c", p=128))
                wtiles[i] = w_t
            fc1b_t = cpool.tile([128, A], F32, name="fc1b_t")
            nc.gpsimd.dma_start(fc1b_t[:], fc1b[:])
            gbc_t = cpool.tile([128, 2 * A], BF16, name="gbc_t")
            nc.gpsimd.dma_start(gbc_t[:], gbc[:])
            gbc32_t = cpool.tile([128, A], F32, name="gbc32_t")
            nc.gpsimd.dma_start(gbc32_t[:], gbc32[:])
            # per-target fc weights are DMA'd mid-run (3 blocks ahead of use)
            fc1w_t, fc2w_t, fc2bias_t = {}, {}, {}

            anchors = {}   # a -> [tile per k]
            adiff = None

            for i in range(NUM_BLOCKS):
                t_idx = target_of.get(i)
                a_idx = anchor_of.get(i)

                # prefetch block weights four blocks out (both HWDGE queues)
                if i + 4 < NUM_BLOCKS and (i + 4) not in wtiles:
                    w_n = wpool.tile([128, KT * C], BF16, tag="w", name=f"w{i+4}")
                    eng = nc.sync if i % 2 == 0 else nc.scalar
                    eng.dma_start(w_n[:].rearrange("p (k c) -> p k c", k=KT),
                                  wd[i + 4].rearrange("(k p) c -> p k c", p=128))
                    wtiles[i + 4] = w_n

                # prefetch the router weights for a target ~3 blocks out
                if i + 3 in target_of:
                    tt = target_of[i + 3]
                    f1 = cpool.tile([128, KT * 128], BF16, name=f"fc1w_{tt}")
                    nc.gpsimd.dma_start(
                        f1[:], fc1w[:, tt * KT * 128:(tt + 1) * KT * 128])
                    fc1w_t[tt] = f1
                    f2 = cpool.tile([128, A * C], BF16, name=f"fc2w_{tt}")
                    nc.gpsimd.dma_start(
                        f2[:], fc2w[:, tt * A * C:(tt + 1) * A * C])
                    fc2w_t[tt] = f2
                    fb = cpool.tile([128, A * KT * BL], F32, name=f"fc2b_{tt}")
                    nc.gpsimd.dma_start(
                        fb[:], fc2bias[:, tt * A * KT * BL:(tt + 1) * A * KT * BL])
                    fc2bias_t[tt] = fb

                w_t = wtiles.pop(i)

                Xn = []
                for ct in range(KT):
                    if a_idx is not None:
                        xn = apool.tile([128, T], BF16, tag=f"a{a_idx}_{ct}",
                                        name=f"anc{a_idx}_{ct}")
                    else:
                        xn = xpool.tile([128, T], BF16, tag=f"x{ct}",
                                        name=f"xb{i}_{ct}")
                    Xn.append(xn)

                if i in POST_ROUTER:
                    # sample-major: matmuls restart per-sample as the router's
                    # corrections land; N=196 groups, gelu per (ct, pair)
                    for bp in range(2):
                        pss = []
                        for ct in range(KT):
                            ps = ppool.tile([128, 512], F32, tag="mm",
                                            name=f"ps{i}_{ct}_{bp}")
                            pss.append(ps)
                        for bi in range(2):
                            b = 2 * bp + bi
                            for ct in range(KT):
                                for k in range(KT):
                                    nc.tensor.matmul(
                                        pss[ct][:, bi * HW:(bi + 1) * HW],
                                        w_t[:, k * C + ct * 128:k * C + (ct + 1) * 128],
                                        X[k][:, b * HW:(b + 1) * HW],
                                        start=(k == 0), stop=(k == KT - 1))
                        for ct in range(KT):
                            nc.scalar.activation(
                                Xn[ct][:, bp * CH:(bp + 1) * CH],
                                pss[ct][:, 0:CH], GELU,
                                bias=bias_t[:, i * KT + ct:i * KT + ct + 1])
                else:
                    # target (router) blocks run ct-major so the router's
                    # per-sample base slices pipeline right behind the gelus;
                    # other blocks run chunk-major
                    if t_idx is not None:
                        order = [(c, ct) for ct in range(KT) for c in range(NCH)]
                    else:
                        order = [(c, ct) for c in range(NCH) for ct in range(KT)]
                    for c, ct in order:
                        ps = ppool.tile([128, 512], F32, tag="mm",
                                        name=f"ps{i}_{ct}_{c}")
                        for k in range(KT):
                            nc.tensor.matmul(
                                ps[:, 0:CH],
                                w_t[:, k * C + ct * 128:k * C + (ct + 1) * 128],
                                X[k][:, c * CH:(c + 1) * CH],
                                start=(k == 0), stop=(k == KT - 1))
                        nc.scalar.activation(
                            Xn[ct][:, c * CH:(c + 1) * CH], ps[:, 0:CH], GELU,
                            bias=bias_t[:, i * KT + ct:i * KT + ct + 1])

                if a_idx is not None:
                    anchors[a_idx] = Xn
                    if a_idx == 2:
                        # precompute per-sample pooled sums of a2 (f32) -- the
                        # routers' mean-pool is recovered from the base-term
                        # accumulators minus gamma * these; DVE is idle here
                        pa2 = []
                        for k in range(KT):
                            p2 = rpool.tile([128, BL], F32, tag=f"pa2_{k}",
                                            name=f"pa2_{k}")
                            nc.vector.reduce_sum(
                                p2[:],
                                Xn[k][:].rearrange("p (b m) -> p b m", b=BL),
                                axis=mybir.AxisListType.X)
                            pa2.append(p2)
                        # precompute anchor differences (gates sum to gamma:
                        # routed = gamma*a2 + g0*(a0-a2) + g1*(a1-a2)) IN
                        # PLACE over a0/a1, whose raw values are dead now
                        adiff = {}
                        for da in range(2):
                            adiff[da] = []
                            for k in range(KT):
                                dt_ = anchors[da][k]
                                nc.vector.tensor_sub(dt_[:], dt_[:],
                                                     anchors[2][k][:])
                                adiff[da].append(dt_)
                if t_idx is not None:
                    Xn = _routing(nc, rpool, xpool, fcps, dpool, t_idx, Xn,
                                  anchors, adiff, pa2, fc1w_t, fc1b_t, fc2w_t,
                                  fc2bias_t, gbc_t, gbc32_t, zw, zx,
                                  outT if i == NUM_BLOCKS - 1 else None)
                X = Xn

    nc.compile()
    return nc


_dummy_ctr = [0]


def _dummy_mms(nc, pool, zw, zx, n512):
    """Dependency-free matmuls on zeroed tiles: keep the PE HAM clock-gate
    warm across windows where real matmuls are blocked. All matmuls write the
    SAME tile (own pool, own bank) so no semaphore round-trips or false deps."""
    _dummy_ctr[0] += 1
    ps = pool.tile([128, 512], F32, tag="dum", name=f"dummy{_dummy_ctr[0]}")
    for j in range(n512):
        nc.tensor.matmul(ps[:, 0:512], zw[:], zx[:], start=True, stop=True)


def _routing(nc, rpool, xpool, fcps, dpool, t, Xn, anchors, adiff, pa2,
             fc1w_t, fc1b_t, fc2w_t, fc2bias_t, gbc_t, gbc32_t, zw, zx,
             outT=None):
    """ChannelGating router: mean-pool -> 2-layer MLP -> softmax over anchors
    -> weighted anchor sum added to Xn. Returns the updated activation tiles."""
    mul = mybir.AluOpType.mult
    add = mybir.AluOpType.add

    # keep HAM warm through the pool/MLP stretch (PE idles right after the
    # block's own matmuls end); not needed after the last block
    if outT is None:
        _dummy_mms(nc, dpool, zw, zx, n512=8)

    # base term of the update (xr = Xn + gamma*a2) is gate-independent; emit
    # it per-sample with accumulators so the mean-pool comes for free:
    # sum(Xn) = accum(xr_base) - gamma * pa2.  (1/196 is folded into fc1w.)
    Xr = []
    pooled = []
    for k in range(KT):
        xr = xpool.tile([128, T], BF16, tag=f"x{k}", name=f"xr{t}_{k}")
        pb = rpool.tile([128, BL], F32, tag=f"pb{k}", name=f"pb{t}_{k}")
        for b in range(BL):
            sl = slice(b * HW, (b + 1) * HW)
            nc.vector.scalar_tensor_tensor(
                xr[:, sl], anchors[2][k][:, sl], gbc_t[:, t:t + 1],
                Xn[k][:, sl], op0=mul, op1=add,
                accum_out=pb[:, b:b + 1])
        pl = rpool.tile([128, BL], BF16, tag=f"pool{k}", name=f"pool{t}_{k}")
        with nc.allow_low_precision(reason="pooled rounds to bf16 on write"):
            nc.vector.scalar_tensor_tensor(
                pl[:], pa2[k][:], gbc_t[:, A + t:A + t + 1], pb[:],
                op0=mul, op1=add)
        Xr.append(xr)
        pooled.append(pl)

    # fc1: h = gelu(pooled @ fc1_w + fc1_b)   [HID=128, BL]
    ps1 = fcps.tile([128, BL], F32, tag="fcps", name=f"ps1_{t}")
    for k in range(KT):
        nc.tensor.matmul(ps1[:], fc1w_t[t][:, k * 128:(k + 1) * 128], pooled[k][:],
                         start=(k == 0), stop=(k == KT - 1))
    h = rpool.tile([128, BL], BF16, tag="h", name=f"h_{t}")
    nc.scalar.activation(h[:], ps1[:], GELU, bias=fc1b_t[:, t:t + 1])

    # fc2: logits [A*C, BL] as 12 col-tiles of one [128, 48] psum
    NJ = A * KT  # 12
    ps2 = fcps.tile([128, NJ * BL], F32, tag="fcps", name=f"ps2_{t}")
    for j in range(NJ):
        nc.tensor.matmul(ps2[:, j * BL:(j + 1) * BL],
                         fc2w_t[t][:, j * 128:(j + 1) * 128],
                         h[:], start=True, stop=True)
    logits = rpool.tile([128, NJ * BL], F32, tag="logits", name=f"lg_{t}")
    nc.vector.tensor_add(logits[:], ps2[:], fc2bias_t[t][:])

    # keep HAM warm across the gate bubble (not needed after the last block):
    # a couple of free dummies now, then micro-matmuls gated on the softmax
    # intermediates so PE activity is spread across the whole bubble
    if outT is None:
        _dummy_mms(nc, dpool, zw, zx, n512=2)

    # softmax over a (cols = a*16 + k*4 + b), exp via tanh identity:
    # e^x = (1 + tanh(x/2)) / (1 - tanh(x/2)); logits are O(0.1) here so
    # the max-subtraction is skipped (tanh path is stable to |x|~17)
    KB = KT * BL  # 16
    th = rpool.tile([128, A * KB], F32, tag="th", name=f"th_{t}")
    nc.scalar.activation(th[:], logits[:], TANH, scale=0.5)
    den = rpool.tile([128, A * KB], F32, tag="den", name=f"den_{t}")
    nc.vector.tensor_scalar(den[:], th[:], -1.0, 1.0, op0=mul, op1=add)
    rec = rpool.tile([128, A * KB], F32, tag="rec", name=f"rec_{t}")
    nc.vector.reciprocal(rec[:], den[:])
    e = rpool.tile([128, A * KB], F32, tag="e", name=f"e_{t}")
    nc.vector.tensor_scalar(e[:], rec[:], 2.0, -1.0, op0=mul, op1=add)
    s = rpool.tile([128, KB], F32, tag="s", name=f"s_{t}")
    nc.vector.tensor_reduce(s[:], e[:].rearrange("p (a kb) -> p kb a", a=A),
                            axis=mybir.AxisListType.X, op=add)
    if outT is None:
        # HAM keep-alive: tiny f32 matmuls that cannot start before th/e
        # exist, landing PE activity in the middle of the gate bubble
        _dummy_ctr[0] += 1
        dps = dpool.tile([128, 512], F32, tag="dum", name=f"dummy{_dummy_ctr[0]}")
        for j in range(2):
            nc.tensor.matmul(dps[0:48, 0:48], th[:, 0:48], logits[:, 0:48],
                             start=True, stop=True)
        for j in range(2):
            nc.tensor.matmul(dps[0:48, 0:48], e[:, 0:48], logits[:, 0:48],
                             start=True, stop=True)
    rinv = rpool.tile([128, KB], F32, tag="rinv", name=f"rinv_{t}")
    nc.vector.reciprocal(rinv[:], s[:])
    rg = rpool.tile([128, KB], F32, tag="rg", name=f"rg_{t}")
    nc.vector.tensor_scalar_mul(rg[:], rinv[:], gbc32_t[:, t:t + 1])
    # gates bf16 (STT scalars) + f32 copy (ACT Identity scale operand)
    g = rpool.tile([128, 2 * KB], BF16, tag="g", name=f"g_{t}")
    g32 = rpool.tile([128, 2 * KB], F32, tag="g32", name=f"g32_{t}")
    with nc.allow_low_precision(reason="gates round to bf16 on write"):
        for a in range(2):
            nc.vector.tensor_mul(g32[:, a * KB:(a + 1) * KB],
                                 e[:, a * KB:(a + 1) * KB], rg[:])
            nc.vector.tensor_mul(g[:, a * KB:(a + 1) * KB],
                                 e[:, a * KB:(a + 1) * KB], rg[:])

    # per-sample corrections, sample-major so the next block restarts per-b:
    # xr[:, b] += g0*(a0-a2) + g1*(a1-a2).  The DVE STT is stuck in 1x mode,
    # so for some k the scalar engine computes tmp = g*diff (Identity with a
    # per-partition scale) and the DVE only does a 2x-mode tensor add.  The
    # last block has no gelu work competing for ACT, so it assists more.
    n_assist = 4                          # k-tiles handled via ACT per (b)
    for b in range(BL):
        sl = slice(b * HW, (b + 1) * HW)
        for k in range(KT):
            for a in range(2):
                col = a * KB + k * BL + b
                if k >= KT - n_assist // 2:
                    tmp = rpool.tile([128, HW], BF16, tag=f"tmp{(2 * k + a) % 4}",
                                     name=f"tmp{t}_{b}_{k}_{a}")
                    nc.scalar.activation(tmp[:], adiff[a][k][:, sl], IDENT,
                                         scale=g32[:, col:col + 1])
                    nc.vector.tensor_add(Xr[k][:, sl], Xr[k][:, sl], tmp[:])
                else:
                    nc.vector.scalar_tensor_tensor(
                        Xr[k][:, sl], adiff[a][k][:, sl],
                        g[:, col:col + 1], Xr[k][:, sl], op0=mul, op1=add)
            if outT is not None and (b == 1 or b == 2):
                # stream the final output as it settles, on both queues:
                # samples 0-1 as one piece, then 2 and 3 individually so the
                # last transfer is as small as possible
                eng = nc.sync if k % 2 == 0 else nc.scalar
                hl = slice((b - 1) * HW, (b + 1) * HW) if b == 1 else sl
                eng.dma_start(outT[k * 128:(k + 1) * 128, hl], Xr[k][:, hl])
            elif outT is not None and b == 3:
                eng = nc.sync if k % 2 == 0 else nc.scalar
                eng.dma_start(outT[k * 128:(k + 1) * 128, sl], Xr[k][:, sl])
    return Xr


def _prep_shared(block_w, block_b, fc1_w, fc1_b, fc2_w, fc2_b, gammas):
    """Host-side packing of the (replicated) weight tensors."""
    import ml_dtypes
    f = np.float32
    bf = ml_dtypes.bfloat16
    wd = np.ascontiguousarray(np.asarray(block_w, dtype=f).astype(bf))
    # bias column (i*KT+ct) = block_b[i, ct*128:(ct+1)*128]
    bias_cols = np.ascontiguousarray(
        np.asarray(block_b, dtype=f).reshape(NUM_BLOCKS * KT, 128).T, dtype=f)
    # fc1 with the mean-pool divisor folded in; col block (t*KT+k)
    fc1s = (np.asarray(fc1_w, dtype=f) / float(HW)).astype(f)   # [A, C, HID]
    fc1w_cat = np.concatenate(
        [fc1s[t][k * 128:(k + 1) * 128, :] for t in range(A) for k in range(KT)],
        axis=1)                                               # [128, A*KT*128]
    fc1b_cols = np.ascontiguousarray(np.asarray(fc1_b, dtype=f).T)  # [128, A]
    fc2w_cat = np.concatenate([np.asarray(fc2_w[t], dtype=f) for t in range(A)],
                              axis=1)                          # [128, A*A*C]
    # fc2 bias expanded to the [128, (a,k,b)] logits layout, repeated per b
    fc2bias = np.concatenate(
        [np.repeat(np.asarray(fc2_b[t], dtype=f).reshape(A * KT, 128).T,
                   BL, axis=1) for t in range(A)], axis=1)     # [128, A*A*KT*BL]
    gam = np.asarray(gammas, dtype=f)
    gbc = np.broadcast_to(np.concatenate([gam, -gam])[None, :], (128, 2 * A))
    gbc = np.ascontiguousarray(gbc.astype(bf))
    gbc32 = np.ascontiguousarray(np.broadcast_to(gam[None, :], (128, A)))
    return dict(wd=wd, bias_cols=bias_cols, gbc32=gbc32,
                fc1w=np.ascontiguousarray(fc1w_cat.astype(bf)),
                fc1b=fc1b_cols,
                fc2w=np.ascontiguousarray(fc2w_cat.astype(bf)),
                fc2bias=np.ascontiguousarray(fc2bias), gbc=gbc)


def shard_x(x):
    """Full x [B,H,W,C] -> per-core transposed shards [C, T] bf16."""
    import ml_dtypes
    shards = []
    for r in range(N_CORES):
        xs = np.asarray(x[r * BL:(r + 1) * BL], dtype=np.float32)  # [BL,H,W,C]
        shards.append(np.ascontiguousarray(
            xs.reshape(T, C).T.astype(ml_dtypes.bfloat16)))        # [C, T]
    return shards


def unshard_out(outs):
    """Per-core [C, T] results -> full [B,H,W,C]."""
    parts = [np.asarray(o, dtype=np.float32).T.reshape(BL, H, W, C)
             for o in outs]
    return np.ascontiguousarray(np.concatenate(parts, axis=0), dtype=np.float32)


def kernel(x, block_w, block_b, fc1_w, fc1_b, fc2_w, fc2_b, gammas):
    if "nc" not in _cached:
        _cached["nc"] = build_program()
    nc = _cached["nc"]

    shared = _prep_shared(block_w, block_b, fc1_w, fc1_b, fc2_w, fc2_b, gammas)
    xs = shard_x(x)
    in_maps = [dict(shared, xT=xs[r]) for r in range(N_CORES)]
    res = run_bass_kernel_spmd(nc, in_maps, list(range(N_CORES)))
    return unshard_out([res.results[r]["outT"] for r in range(N_CORES)])


# revision 29
# speedup vs baseline: 1.0069x; 1.0069x over previous
"""Trainium2 Bass kernel for DynamicResidualStageWrapper (18-block MLP stage
with channel-gated anchor routing), data-parallel over batch across 8 cores.

Contract: kernel(**inputs) takes FULL unsharded inputs (as numpy arrays, keyed
as in reference.setup_inputs()) and returns the FULL output [32,14,14,512].

Per-core layout: activations live transposed as [C=512, tokens=784] split into
4 partition-tiles [128, 784] bf16; tokens are (sample b, position hw). Block
weights [cin, cout] are the natural lhsT for out[cout, tok] = W.T @ X so there
are no transposes anywhere. Everything runs in bf16 (tolerance 2e-2; measured
~5e-3): same PE rate as f32r but half the DMA/SBUF traffic and 2x DVE rate.

Key scheduling ideas vs a straightforward version:
- startup: ACT table loads fire before the input-DMA flood (else they starve
  behind it and block every block-0 gelu); descriptor generation is ~0.7us
  serial per dma_start, so transfers are consolidated (one rearranged-AP call
  per weight block) and spread over the sync/scalar/gpsimd queues; dummy
  matmuls on zeroed tiles warm the PE HAM clock-gate so real matmuls run at
  2.4 GHz from the first block.
- routers: the gate-independent base term (Xn + gamma*a2) is emitted as
  per-sample slices whose accumulators give the mean-pool for free (corrected
  by precomputed gamma*pooled(a2)); corrections are applied sample-major so
  the next block's matmuls (sliced per-sample, N=196) restart as soon as the
  first sample's gates land; the DVE's scalar_tensor_tensor is stuck in
  1x mode, so half of each sample's corrections run as ACT Identity
  (per-partition gate scale) + 2x-mode DVE adds. Dummy matmuls keep HAM warm
  across the router bubble.
- block 17 streams its output to HBM per (k, sample-pair) as corrections
  complete, across both HWDGE queues, shrinking the tail.
"""

import numpy as np

import concourse.bacc as bacc
import concourse.mybir as mybir
import concourse.tile as tile
from concourse.bass_utils import run_bass_kernel_spmd

# ---- problem constants (hardcoded per spec) ----
NUM_BLOCKS = 18
ANCHOR_IDX = (1, 4, 9)
TARGET_IDX = (11, 14, 17)
POST_ROUTER = (12, 15)     # blocks whose input trickles in per-sample
C = 512
HID = 128
A = 3
B, H, W = 32, 14, 14
N_CORES = 8
BL = B // N_CORES          # 4 samples per core
HW = H * W                 # 196 positions per sample
T = BL * HW                # 784 tokens per core
KT = C // 128              # 4 channel tiles
NCH = 2                    # token chunks per row (normal blocks)
CH = T // NCH              # 392 tokens per chunk

F32 = mybir.dt.float32
BF16 = mybir.dt.bfloat16
GELU = mybir.ActivationFunctionType.Gelu_apprx_tanh
TANH = mybir.ActivationFunctionType.Tanh
IDENT = mybir.ActivationFunctionType.Identity
_cached = {}


def build_program():
    """Build the per-core Bass/Tile program (same program on all 8 cores)."""
    nc = bacc.Bacc(trn_type="TRN2", target_bir_lowering=False, debug=False)

    xT = nc.dram_tensor("xT", [C, T], BF16, kind="ExternalInput").ap()
    wd = nc.dram_tensor("wd", [NUM_BLOCKS, C, C], BF16, kind="ExternalInput").ap()
    bias_cols = nc.dram_tensor("bias_cols", [128, NUM_BLOCKS * KT], F32,
                               kind="ExternalInput").ap()
    fc1w = nc.dram_tensor("fc1w", [128, A * KT * 128], BF16, kind="ExternalInput").ap()
    fc1b = nc.dram_tensor("fc1b", [128, A], F32, kind="ExternalInput").ap()
    fc2w = nc.dram_tensor("fc2w", [128, A * A * C], BF16, kind="ExternalInput").ap()
    fc2bias = nc.dram_tensor("fc2bias", [128, A * A * KT * BL], F32,
                             kind="ExternalInput").ap()
    # cols 0..A-1: gamma_t; cols A..2A-1: -gamma_t (bf16 so DVE scalar-tensor
    # ops keep their 2x bf16 mode); f32 copy for ops that require f32 scalars
    gbc = nc.dram_tensor("gbc", [128, 2 * A], BF16, kind="ExternalInput").ap()
    gbc32 = nc.dram_tensor("gbc32", [128, A], F32, kind="ExternalInput").ap()
    outT = nc.dram_tensor("outT", [C, T], BF16, kind="ExternalOutput").ap()

    anchor_of = {b: i for i, b in enumerate(ANCHOR_IDX)}
    target_of = {b: i for i, b in enumerate(TARGET_IDX)}

    with tile.TileContext(nc) as tc:
        with (
            tc.tile_pool(name="const", bufs=1) as cpool,
            tc.tile_pool(name="wpool", bufs=6) as wpool,
            tc.tile_pool(name="xpool", bufs=4) as xpool,
            tc.tile_pool(name="apool", bufs=1) as apool,
            tc.tile_pool(name="rpool", bufs=2) as rpool,
            tc.tile_pool(name="ppool", bufs=6, space="PSUM") as ppool,
            tc.tile_pool(name="fcps", bufs=1, space="PSUM") as fcps,
            tc.tile_pool(name="dpool", bufs=1, space="PSUM") as dpool,
        ):
            # ---- HAM warm-up: zeroed bf16 tiles + dummy matmuls keep the PE
            # busy during the startup DMA wait so real matmuls start at 2.4GHz
            # (memsets on gpsimd: its queue starts ~1us before the vector one)
            zw = cpool.tile([128, 128], BF16, name="zw")
            zx = cpool.tile([128, 512], BF16, name="zx")
            # gelu ACT-table load MUST fire before the input-DMA flood starts,
            # or its TDRAM DMA gets starved and blocks every block-0 gelu.
            # zw/zx memsets go on the (otherwise idle) vector engine so the
            # gpsimd queue reaches its x-piece descriptor gens sooner.
            warm = cpool.tile([128, 1], F32, name="warm")
            nc.gpsimd.memset(warm[:], 0.0)
            nc.scalar.activation(warm[:], warm[:], GELU)
            nc.vector.memset(zw[:], 0.0)
            nc.vector.memset(zx[:], 0.0)
            _dummy_mms(nc, dpool, zw, zx, n512=4)

            # ---- startup DMAs.  Descriptor generation costs ~0.7us SERIAL
            # per dma_start on the issuing queue, so: the scalar/ACT queue
            # does ONLY the table warms (+tiny bias), the sync queue carries
            # w0/w1 + two x chunk-0 pieces, gpsimd (SWDGE) the other two.
            nc.scalar.activation(warm[:], warm[:], TANH)
            nc.scalar.activation(warm[:], warm[:], IDENT)
            X = []
            for k in range(KT):
                xt = xpool.tile([128, T], BF16, tag=f"x{k}", name=f"xin{k}")
                X.append(xt)
            wtiles = {}
            w_t = wpool.tile([128, KT * C], BF16, tag="w", name="w0")
            nc.sync.dma_start(w_t[:].rearrange("p (k c) -> p k c", k=KT),
                              wd[0].rearrange("(k p) c -> p k c", p=128))
            wtiles[0] = w_t
            for k in range(KT):
                eng = nc.sync if k < 2 else nc.gpsimd
                eng.dma_start(X[k][:, 0:CH], xT[k * 128:(k + 1) * 128, 0:CH])
            bias_t = cpool.tile([128, NUM_BLOCKS * KT], F32, name="bias_t")
            nc.scalar.dma_start(bias_t[:], bias_cols[:])
            w_t = wpool.tile([128, KT * C], BF16, tag="w", name="w1")
            nc.sync.dma_start(w_t[:].rearrange("p (k c) -> p k c", k=KT),
                              wd[1].rearrange("(k p) c -> p k c", p=128))
            wtiles[1] = w_t
            for k in range(KT):
                eng = nc.sync if k < 2 else nc.gpsimd
                eng.dma_start(X[k][:, CH:T], xT[k * 128:(k + 1) * 128, CH:T])
            for i in (2, 3):
                w_t = wpool.tile([128, KT * C], BF16, tag="w", name=f"w{i}")
                eng = nc.sync if i == 2 else nc.scalar
                eng.dma_start(w_t[:].rearrange("p (k c) -> p k c", k=KT),
                              wd[i].rearrange("(k p) c -> p k c", p=128))
                wtiles[i] = w_t
            fc1b_t = cpool.tile([128, A], F32, name="fc1b_t")
            nc.gpsimd.dma_start(fc1b_t[:], fc1b[:])
            gbc_t = cpool.tile([128, 2 * A], BF16, name="gbc_t")
            nc.gpsimd.dma_start(gbc_t[:], gbc[:])
            gbc32_t = cpool.tile([128, A], F32, name="gbc32_t")
            nc.gpsimd.dma_start(gbc32_t[:], gbc32[:])
            # per-target fc weights are DMA'd mid-run (3 blocks ahead of use)
            fc1w_t, fc2w_t, fc2bias_t = {}, {}, {}

            anchors = {}   # a -> [tile per k]
            adiff = None

            for i in range(NUM_BLOCKS):
                t_idx = target_of.get(i)
                a_idx = anchor_of.get(i)

                # prefetch block weights four blocks out (both HWDGE queues)
                if i + 4 < NUM_BLOCKS and (i + 4) not in wtiles:
                    w_n = wpool.tile([128, KT * C], BF16, tag="w", name=f"w{i+4}")
                    eng = nc.sync if i % 2 == 0 else nc.scalar
                    eng.dma_start(w_n[:].rearrange("p (k c) -> p k c", k=KT),
                                  wd[i + 4].rearrange("(k p) c -> p k c", p=128))
                    wtiles[i + 4] = w_n

                # prefetch the router weights for a target ~3 blocks out
                if i + 3 in target_of:
                    tt = target_of[i + 3]
                    f1 = cpool.tile([128, KT * 128], BF16, name=f"fc1w_{tt}")
                    nc.gpsimd.dma_start(
                        f1[:], fc1w[:, tt * KT * 128:(tt + 1) * KT * 128])
                    fc1w_t[tt] = f1
                    f2 = cpool.tile([128, A * C], BF16, name=f"fc2w_{tt}")
                    nc.gpsimd.dma_start(
                        f2[:], fc2w[:, tt * A * C:(tt + 1) * A * C])
                    fc2w_t[tt] = f2
                    fb = cpool.tile([128, A * KT * BL], F32, name=f"fc2b_{tt}")
                    nc.gpsimd.dma_start(
                        fb[:], fc2bias[:, tt * A * KT * BL:(tt + 1) * A * KT * BL])
                    fc2bias_t[tt] = fb

                w_t = wtiles.pop(i)

                Xn = []
                for ct in range(KT):
                    if a_idx is not None:
                        xn = apool.tile([128, T], BF16, tag=f"a{a_idx}_{ct}",
                                        name=f"anc{a_idx}_{ct}")
                    else:
                        xn = xpool.tile([128, T], BF16, tag=f"x{ct}",
                                        name=f"xb{i}_{ct}")
                    Xn.append(xn)

                if i in POST_ROUTER:
                    # sample-major: matmuls restart per-sample as the router's
                    # corrections land; N=196 groups, gelu per (ct, pair)
                    for bp in range(2):
                        pss = []
                        for ct in range(KT):
                            ps = ppool.tile([128, 512], F32, tag="mm",
                                            name=f"ps{i}_{ct}_{bp}")
                            pss.append(ps)
                        for bi in range(2):
                            b = 2 * bp + bi
                            for ct in range(KT):
                                for k in range(KT):
                                    nc.tensor.matmul(
                                        pss[ct][:, bi * HW:(bi + 1) * HW],
                                        w_t[:, k * C + ct * 128:k * C + (ct + 1) * 128],
                                        X[k][:, b * HW:(b + 1) * HW],
                                        start=(k == 0), stop=(k == KT - 1))
                        for ct in range(KT):
                            nc.scalar.activation(
                                Xn[ct][:, bp * CH:(bp + 1) * CH],
                                pss[ct][:, 0:CH], GELU,
                                bias=bias_t[:, i * KT + ct:i * KT + ct + 1])
                else:
                    # target (router) blocks run ct-major so the router's
                    # per-sample base slices pipeline right behind the gelus;
                    # other blocks run chunk-major
                    if t_idx is not None:
                        order = [(c, ct) for ct in range(KT) for c in range(NCH)]
                    else:
                        order = [(c, ct) for c in range(NCH) for ct in range(KT)]
                    for c, ct in order:
                        ps = ppool.tile([128, 512], F32, tag="mm",
                                        name=f"ps{i}_{ct}_{c}")
                        for k in range(KT):
                            nc.tensor.matmul(
                                ps[:, 0:CH],
                                w_t[:, k * C + ct * 128:k * C + (ct + 1) * 128],
                                X[k][:, c * CH:(c + 1) * CH],
                                start=(k == 0), stop=(k == KT - 1))
                        nc.scalar.activation(
                            Xn[ct][:, c * CH:(c + 1) * CH], ps[:, 0:CH], GELU,
                            bias=bias_t[:, i * KT + ct:i * KT + ct + 1])

                if a_idx is not None:
                    anchors[a_idx] = Xn
                    if a_idx == 2:
                        # precompute per-sample pooled sums of a2 (f32) -- the
                        # routers' mean-pool is recovered from the base-term
                        # accumulators minus gamma * these; DVE is idle here
                        pa2 = []
                        for k in range(KT):
                            p2 = rpool.tile([128, BL], F32, tag=f"pa2_{k}",
                                            name=f"pa2_{k}")
                            nc.vector.reduce_sum(
                                p2[:],
                                Xn[k][:].rearrange("p (b m) -> p b m", b=BL),
                                axis=mybir.AxisListType.X)
                            pa2.append(p2)
                        # precompute anchor differences (gates sum to gamma:
                        # routed = gamma*a2 + g0*(a0-a2) + g1*(a1-a2)) IN
                        # PLACE over a0/a1, whose raw values are dead now
                        adiff = {}
                        for da in range(2):
                            adiff[da] = []
                            for k in range(KT):
                                dt_ = anchors[da][k]
                                nc.vector.tensor_sub(dt_[:], dt_[:],
                                                     anchors[2][k][:])
                                adiff[da].append(dt_)
                if t_idx is not None:
                    Xn = _routing(nc, rpool, xpool, fcps, dpool, t_idx, Xn,
                                  anchors, adiff, pa2, fc1w_t, fc1b_t, fc2w_t,
                                  fc2bias_t, gbc_t, gbc32_t, zw, zx,
                                  outT if i == NUM_BLOCKS - 1 else None)
                X = Xn

    nc.compile()
    return nc


_dummy_ctr = [0]


def _dummy_mms(nc, pool, zw, zx, n512):
    """Dependency-free matmuls on zeroed tiles: keep the PE HAM clock-gate
    warm across windows where real matmuls are blocked. All matmuls write the
    SAME tile (own pool, own bank) so no semaphore round-trips or false deps."""
    _dummy_ctr[0] += 1
    ps = pool.tile([128, 512], F32, tag="dum", name=f"dummy{_dummy_ctr[0]}")
    for j in range(n512):
        nc.tensor.matmul(ps[:, 0:512], zw[:], zx[:], start=True, stop=True)


def _routing(nc, rpool, xpool, fcps, dpool, t, Xn, anchors, adiff, pa2,
             fc1w_t, fc1b_t, fc2w_t, fc2bias_t, gbc_t, gbc32_t, zw, zx,
             outT=None):
    """ChannelGating router: mean-pool -> 2-layer MLP -> softmax over anchors
    -> weighted anchor sum added to Xn. Returns the updated activation tiles."""
    mul = mybir.AluOpType.mult
    add = mybir.AluOpType.add

    # keep HAM warm through the pool/MLP stretch (PE idles right after the
    # block's own matmuls end); not needed after the last block
    if outT is None:
        _dummy_mms(nc, dpool, zw, zx, n512=8)

    # base term of the update (xr = Xn + gamma*a2) is gate-independent; emit
    # it per-sample with accumulators so the mean-pool comes for free:
    # sum(Xn) = accum(xr_base) - gamma * pa2.  (1/196 is folded into fc1w.)
    Xr = []
    pooled = []
    for k in range(KT):
        xr = xpool.tile([128, T], BF16, tag=f"x{k}", name=f"xr{t}_{k}")
        pb = rpool.tile([128, BL], F32, tag=f"pb{k}", name=f"pb{t}_{k}")
        for b in range(BL):
            sl = slice(b * HW, (b + 1) * HW)
            nc.vector.scalar_tensor_tensor(
                xr[:, sl], anchors[2][k][:, sl], gbc_t[:, t:t + 1],
                Xn[k][:, sl], op0=mul, op1=add,
                accum_out=pb[:, b:b + 1])
        pl = rpool.tile([128, BL], BF16, tag=f"pool{k}", name=f"pool{t}_{k}")
        with nc.allow_low_precision(reason="pooled rounds to bf16 on write"):
            nc.vector.scalar_tensor_tensor(
                pl[:], pa2[k][:], gbc_t[:, A + t:A + t + 1], pb[:],
                op0=mul, op1=add)
        Xr.append(xr)
        pooled.append(pl)

    # fc1: h = gelu(pooled @ fc1_w + fc1_b)   [HID=128, BL]
    ps1 = fcps.tile([128, BL], F32, tag="fcps", name=f"ps1_{t}")
    for k in range(KT):
        nc.tensor.matmul(ps1[:], fc1w_t[t][:, k * 128:(k + 1) * 128], pooled[k][:],
                         start=(k == 0), stop=(k == KT - 1))
    h = rpool.tile([128, BL], BF16, tag="h", name=f"h_{t}")
    nc.scalar.activation(h[:], ps1[:], GELU, bias=fc1b_t[:, t:t + 1])

    # fc2: logits [A*C, BL] as 12 col-tiles of one [128, 48] psum
    NJ = A * KT  # 12
    ps2 = fcps.tile([128, NJ * BL], F32, tag="fcps", name=f"ps2_{t}")
    for j in range(NJ):
        nc.tensor.matmul(ps2[:, j * BL:(j + 1) * BL],
                         fc2w_t[t][:, j * 128:(j + 1) * 128],
                         h[:], start=True, stop=True)
    logits = rpool.tile([128, NJ * BL], F32, tag="logits", name=f"lg_{t}")
    nc.vector.tensor_add(logits[:], ps2[:], fc2bias_t[t][:])

    # keep HAM warm across the gate bubble (not needed after the last block)
    if outT is None:
        _dummy_mms(nc, dpool, zw, zx, n512=8)

    # softmax over a (cols = a*16 + k*4 + b), exp via tanh identity:
    # e^x = (1 + tanh(x/2)) / (1 - tanh(x/2)); logits are O(0.1) here so
    # the max-subtraction is skipped (tanh path is stable to |x|~17)
    KB = KT * BL  # 16
    th = rpool.tile([128, A * KB], F32, tag="th", name=f"th_{t}")
    nc.scalar.activation(th[:], logits[:], TANH, scale=0.5)
    den = rpool.tile([128, A * KB], F32, tag="den", name=f"den_{t}")
    nc.vector.tensor_scalar(den[:], th[:], -1.0, 1.0, op0=mul, op1=add)
    rec = rpool.tile([128, A * KB], F32, tag="rec", name=f"rec_{t}")
    nc.vector.reciprocal(rec[:], den[:])
    e = rpool.tile([128, A * KB], F32, tag="e", name=f"e_{t}")
    nc.vector.tensor_scalar(e[:], rec[:], 2.0, -1.0, op0=mul, op1=add)
    s = rpool.tile([128, KB], F32, tag="s", name=f"s_{t}")
    nc.vector.tensor_reduce(s[:], e[:].rearrange("p (a kb) -> p kb a", a=A),
                            axis=mybir.AxisListType.X, op=add)
    rinv = rpool.tile([128, KB], F32, tag="rinv", name=f"rinv_{t}")
    nc.vector.reciprocal(rinv[:], s[:])
    rg = rpool.tile([128, KB], F32, tag="rg", name=f"rg_{t}")
    nc.vector.tensor_scalar_mul(rg[:], rinv[:], gbc32_t[:, t:t + 1])
    # gates bf16 (STT scalars) + f32 copy (ACT Identity scale operand)
    g = rpool.tile([128, 2 * KB], BF16, tag="g", name=f"g_{t}")
    g32 = rpool.tile([128, 2 * KB], F32, tag="g32", name=f"g32_{t}")
    with nc.allow_low_precision(reason="gates round to bf16 on write"):
        for a in range(2):
            nc.vector.tensor_mul(g32[:, a * KB:(a + 1) * KB],
                                 e[:, a * KB:(a + 1) * KB], rg[:])
            nc.vector.tensor_mul(g[:, a * KB:(a + 1) * KB],
                                 e[:, a * KB:(a + 1) * KB], rg[:])

    # per-sample corrections, sample-major so the next block restarts per-b:
    # xr[:, b] += g0*(a0-a2) + g1*(a1-a2).  The DVE STT is stuck in 1x mode,
    # so for some k the scalar engine computes tmp = g*diff (Identity with a
    # per-partition scale) and the DVE only does a 2x-mode tensor add.  The
    # last block has no gelu work competing for ACT, so it assists more.
    n_assist = 4                          # k-tiles handled via ACT per (b)
    for b in range(BL):
        sl = slice(b * HW, (b + 1) * HW)
        for k in range(KT):
            for a in range(2):
                col = a * KB + k * BL + b
                if k >= KT - n_assist // 2:
                    tmp = rpool.tile([128, HW], BF16, tag=f"tmp{(2 * k + a) % 4}",
                                     name=f"tmp{t}_{b}_{k}_{a}")
                    nc.scalar.activation(tmp[:], adiff[a][k][:, sl], IDENT,
                                         scale=g32[:, col:col + 1])
                    nc.vector.tensor_add(Xr[k][:, sl], Xr[k][:, sl], tmp[:])
                else:
                    nc.vector.scalar_tensor_tensor(
                        Xr[k][:, sl], adiff[a][k][:, sl],
                        g[:, col:col + 1], Xr[k][:, sl], op0=mul, op1=add)
            if outT is not None and (b == 1 or b == 2):
                # stream the final output as it settles, on both queues:
                # samples 0-1 as one piece, then 2 and 3 individually so the
                # last transfer is as small as possible
                eng = nc.sync if k % 2 == 0 else nc.scalar
                hl = slice((b - 1) * HW, (b + 1) * HW) if b == 1 else sl
                eng.dma_start(outT[k * 128:(k + 1) * 128, hl], Xr[k][:, hl])
            elif outT is not None and b == 3:
                eng = nc.sync if k % 2 == 0 else nc.scalar
                eng.dma_start(outT[k * 128:(k + 1) * 128, sl], Xr[k][:, sl])
    return Xr


def _prep_shared(block_w, block_b, fc1_w, fc1_b, fc2_w, fc2_b, gammas):
    """Host-side packing of the (replicated) weight tensors."""
    import ml_dtypes
    f = np.float32
    bf = ml_dtypes.bfloat16
    wd = np.ascontiguousarray(np.asarray(block_w, dtype=f).astype(bf))
    # bias column (i*KT+ct) = block_b[i, ct*128:(ct+1)*128]
    bias_cols = np.ascontiguousarray(
        np.asarray(block_b, dtype=f).reshape(NUM_BLOCKS * KT, 128).T, dtype=f)
    # fc1 with the mean-pool divisor folded in; col block (t*KT+k)
    fc1s = (np.asarray(fc1_w, dtype=f) / float(HW)).astype(f)   # [A, C, HID]
    fc1w_cat = np.concatenate(
        [fc1s[t][k * 128:(k + 1) * 128, :] for t in range(A) for k in range(KT)],
        axis=1)                                               # [128, A*KT*128]
    fc1b_cols = np.ascontiguousarray(np.asarray(fc1_b, dtype=f).T)  # [128, A]
    fc2w_cat = np.concatenate([np.asarray(fc2_w[t], dtype=f) for t in range(A)],
                              axis=1)                          # [128, A*A*C]
    # fc2 bias expanded to the [128, (a,k,b)] logits layout, repeated per b
    fc2bias = np.concatenate(
        [np.repeat(np.asarray(fc2_b[t], dtype=f).reshape(A * KT, 128).T,
                   BL, axis=1) for t in range(A)], axis=1)     # [128, A*A*KT*BL]
    gam = np.asarray(gammas, dtype=f)
    gbc = np.broadcast_to(np.concatenate([gam, -gam])[None, :], (128, 2 * A))
    gbc = np.ascontiguousarray(gbc.astype(bf))
    gbc32 = np.ascontiguousarray(np.broadcast_to(gam[None, :], (128, A)))
    return dict(wd=wd, bias_cols=bias_cols, gbc32=gbc32,
                fc1w=np.ascontiguousarray(fc1w_cat.astype(bf)),
                fc1b=fc1b_cols,
                fc2w=np.ascontiguousarray(fc2w_cat.astype(bf)),
                fc2bias=np.ascontiguousarray(fc2bias), gbc=gbc)


def shard_x(x):
    """Full x [B,H,W,C] -> per-core transposed shards [C, T] bf16."""
    import ml_dtypes
    shards = []
    for r in range(N_CORES):
        xs = np.asarray(x[r * BL:(r + 1) * BL], dtype=np.float32)  # [BL,H,W,C]
        shards.append(np.ascontiguousarray(
            xs.reshape(T, C).T.astype(ml_dtypes.bfloat16)))        # [C, T]
    return shards


def unshard_out(outs):
    """Per-core [C, T] results -> full [B,H,W,C]."""
    parts = [np.asarray(o, dtype=np.float32).T.reshape(BL, H, W, C)
             for o in outs]
    return np.ascontiguousarray(np.concatenate(parts, axis=0), dtype=np.float32)


def kernel(x, block_w, block_b, fc1_w, fc1_b, fc2_w, fc2_b, gammas):
    if "nc" not in _cached:
        _cached["nc"] = build_program()
    nc = _cached["nc"]

    shared = _prep_shared(block_w, block_b, fc1_w, fc1_b, fc2_w, fc2_b, gammas)
    xs = shard_x(x)
    in_maps = [dict(shared, xT=xs[r]) for r in range(N_CORES)]
    res = run_bass_kernel_spmd(nc, in_maps, list(range(N_CORES)))
    return unshard_out([res.results[r]["outT"] for r in range(N_CORES)])
